# revision 1
# baseline (speedup 1.0000x reference)
"""Trainium2 Bass kernel for nn_Net_87076166960184 (retrieval_knn).

Reference computation per batch b of 16:
  d[n,m]   = |x_n - x_m|^2 over N=4096 points in R^3
  idx      = 32 nearest neighbors per point (incl. self)
  features = 30 channels of neighbor statistics per point
  out[b]   = max-pool_n(features) @ W.T + b          -> [16, 32]

Strategy (pure data parallelism, 2 batches per core):
  - PE computes closeness scores e[n,m] = 2<x_n,x_m> - |x_m|^2 per 128-row
    tile (K=4 matmul; row max = self; descending order = nearest first).
  - ACT copies PSUM->SBUF; DVE extracts the exact top-32 values + indices
    per row via 4 rounds of (max8 -> max_index -> match_replace).
  - GPSIMD indirect_copy gathers the 32 neighbors' xyz per row.  Indices
    are ordered neighbor-major per 16-partition group, which makes the
    hardware's wrapped (s p) index layout equal to the natural [128,32]
    index tile - no shuffle needed.  Output is 16x redundant within each
    group; the final max-pool collapses the redundancy.
  - Stats (sum/max/min/sum-sq over the 32 neighbors) via segmented DVE
    reduces; 30 feature channels assembled batch-level; max-pool via
    free-dim reduce + partition_all_reduce; tiny linear on PE.
"""

import os
from contextlib import ExitStack

import numpy as np

import concourse.bacc as bacc
import concourse.bass as bass
import concourse.mybir as mybir
import concourse.tile as tile
from concourse import bass_isa
from concourse.bass_utils import run_bass_kernel_spmd

F32 = mybir.dt.float32
U16 = mybir.dt.uint16
AX = mybir.AxisListType
ALU = mybir.AluOpType
ACTF = mybir.ActivationFunctionType

B, N, C = 16, 4096, 3
NCORES = 8
BPC = B // NCORES          # batches per core
KNN = 32
EPS = 1e-8
NEG = -3.0e38


def build_kernel(n_points=N, bpc=BPC, reps=1, no_idx=False, storm_only=False):
    if storm_only:
        no_idx = True
    nrt = n_points // 128                  # row tiles per batch
    nchunk = n_points // 512               # 512-wide matmul chunks
    xel = n_points * C                     # elements of one batch's x

    nc = bacc.Bacc("TRN2", target_bir_lowering=False, debug=False)
    x_in = nc.dram_tensor("xs", [bpc, n_points, C], F32, kind="ExternalInput")
    w_in = nc.dram_tensor("w", [32, 30], F32, kind="ExternalInput")
    b_in = nc.dram_tensor("bias", [1, 32], F32, kind="ExternalInput")
    out_d = nc.dram_tensor("out", [bpc, 32], F32, kind="ExternalOutput")
    pool_scratch = nc.dram_tensor("pool_scratch", [1, 30], F32)

    with tile.TileContext(nc) as tc, ExitStack() as ctx:
        psum = ctx.enter_context(tc.tile_pool(name="psum", bufs=2, space="PSUM"))
        const = ctx.enter_context(tc.tile_pool(name="const", bufs=1))
        epool = ctx.enter_context(tc.tile_pool(name="ebuf", bufs=2))
        spool = ctx.enter_context(tc.tile_pool(name="small", bufs=2))
        gpool = ctx.enter_context(tc.tile_pool(name="gath", bufs=3))
        apool = ctx.enter_context(tc.tile_pool(name="asm", bufs=1))

        wT = const.tile([30, 32], F32, tag="wT")
        nc.sync.dma_start(wT[:], bass.AP(w_in, 0, [[1, 30], [30, 32]]))
        brow = const.tile([1, 32], F32, tag="brow")
        nc.sync.dma_start(brow[:], b_in[:, :])

        ones3 = const.tile([3, 1], F32, tag="ones3")
        nc.vector.memset(ones3[:], 1.0)
        neg1t = const.tile([1, 512], F32, tag="neg1t")
        nc.vector.memset(neg1t[:], -1.0)

        for rep_bi in range(reps * bpc):
            bi = rep_bi % bpc
            xoff = bi * xel
            # rhs4 rows: [x0, x1, x2, sq];  lhsTall rows: [2x0, 2x1, 2x2, -1]
            rhs4 = const.tile([4, n_points], F32, tag="rhs4")
            nc.sync.dma_start(rhs4[0:3, :],
                              bass.AP(x_in, xoff, [[1, 3], [3, n_points]]))
            lhsTall = const.tile([4, n_points], F32, tag="lhsTall")
            nc.scalar.mul(lhsTall[0:3, :], rhs4[0:3, :], 2.0)
            xT2 = epool.tile([128, n_points], F32, tag="e")
            nc.scalar.activation(xT2[0:3, :], rhs4[0:3, :], ACTF.Square)
            for j in range(nchunk):
                ch = slice(j * 512, (j + 1) * 512)
                nc.sync.dma_start(lhsTall[3:4, ch], neg1t[:])
                ps = psum.tile([128, 2048], F32, tag="ps")
                nc.tensor.matmul(
                    ps[0:1, 0:512], ones3[:], xT2[0:3, ch],
                    start=True, stop=True,
                )
                sqt = spool.tile([1, 512], F32, tag="sqt")
                nc.scalar.copy(sqt[:], ps[0:1, 0:512])
                nc.sync.dma_start(rhs4[3:4, ch], sqt[:])

            # per-coordinate x rows replicated to all partitions for the gather
            crep = []
            for c in range(3):
                r = const.tile([128, n_points], F32, tag=f"crep{c}")
                nc.sync.dma_start(
                    r[:], bass.AP(x_in, xoff + c, [[0, 128], [3, n_points]]))
                crep.append(r)

            # x in (core,slot) layout: [128, nrt, 48], x_core[16k+p, rt, s*3+c] = x[rt*128+16k+s, c]
            x_core = const.tile([128, nrt * 48], F32, tag="x_core")
            for k in range(8):
                nc.sync.dma_start(
                    x_core[16 * k:16 * (k + 1), :].rearrange(
                        "p (rt sc) -> p rt sc", sc=48),
                    bass.AP(x_in, xoff + 16 * k * 3, [[0, 16], [384, nrt], [1, 48]]),
                )

            # ---- per-batch stat accumulators ----
            s1_all = const.tile([128, nrt * 48], F32, tag="s1_all")
            s2_all = const.tile([128, nrt * 48], F32, tag="s2_all")
            rmax_all = const.tile([128, nrt * 48], F32, tag="rmax_all")
            rmin_all = const.tile([128, nrt * 48], F32, tag="rmin_all")
            if storm_only:  # keep never-written tiles legal for Tile release
                for t in (s1_all, s2_all, rmax_all, rmin_all):
                    nc.vector.memset(t[:], 1.0)

            # ---- main loop over row tiles (stats deferred one iteration so
            # DVE reduces of rt-1 overlap the GPSIMD gather of rt) ----
            def emit_stats(rt, gbufs):
                sl = slice(rt * 48, (rt + 1) * 48)

                def stat_slot(acc, c):
                    return acc[:, sl].rearrange(
                        "p (s c) -> p c s", c=3)[:, c:c + 1, :]

                for c in range(3):
                    gc = gbufs[c]
                    gj = gc[:].rearrange("p (j s) -> p s j", s=16)
                    nc.vector.tensor_reduce(stat_slot(s1_all, c), gj, axis=AX.X, op=ALU.add)
                    nc.vector.tensor_reduce(stat_slot(rmax_all, c), gj, axis=AX.X, op=ALU.max)
                    nc.vector.tensor_reduce(stat_slot(rmin_all, c), gj, axis=AX.X, op=ALU.min)
                    nc.scalar.activation(gc[:], gc[:], ACTF.Square)
                    nc.vector.tensor_reduce(stat_slot(s2_all, c), gj, axis=AX.X, op=ALU.add)

            pending = []
            for rt in range(nrt):
                ebuf = epool.tile([128, n_points], F32, tag="e")
                for half in range(2):
                    ps = psum.tile([128, 2048], F32, tag="ps")
                    for j in range(4):
                        ch = half * 2048 + j * 512
                        if ch >= n_points:
                            continue
                        nc.tensor.matmul(
                            ps[:, j * 512:(j + 1) * 512],
                            lhsTall[:, rt * 128:(rt + 1) * 128],
                            rhs4[:, ch:ch + 512],
                            start=True, stop=True,
                        )
                    lo = half * 2048
                    hi = min(lo + 2048, n_points)
                    if lo < n_points:
                        nc.scalar.copy(ebuf[:, lo:hi], ps[:, 0:hi - lo])

                vals = spool.tile([128, 32], F32, tag="vals")
                idxs = spool.tile([128, 32], U16, tag="idxs")
                if no_idx:  # timing-attribution variant: wrong results
                    nc.vector.memset(idxs[:], 0)
                for it in range(4):
                    v8 = vals[:, it * 8:(it + 1) * 8]
                    i8 = idxs[:, it * 8:(it + 1) * 8]
                    nc.vector.max(v8, ebuf[:])
                    if not no_idx:
                        nc.vector.max_index(i8, v8, ebuf[:])
                    if it < 3:
                        nc.vector.match_replace(ebuf[:], v8, ebuf[:], NEG)

                if storm_only:
                    continue
                # Per-coordinate gather: gc[16k+p, 16j+s] = x_c[nbr_j(row 16k+s)]
                gbufs = []
                for c in range(3):
                    gc = gpool.tile([128, KNN * 16], F32, tag=f"g{c}")
                    nc.gpsimd.indirect_copy(gc[:], crep[c][:], idxs[:], True)
                    gbufs.append(gc)
                pending.append((rt, gbufs))
                if len(pending) > 2:
                    emit_stats(*pending.pop(0))
            while pending:
                emit_stats(*pending.pop(0))

            # ---- feature assembly, batch level ----
            # layout of all [128, nrt*48] stat tensors: (rt, s, c)
            fmax = apool.tile([128, 30], F32, tag="fmax")
            nf = nrt * 48

            def pool_channel(src_ap, col):
                # src_ap: [128, nf] in (rt, s, c) layout; max over (rt, s) -> [128, 3]
                v = src_ap.rearrange("p (rt s c) -> p c rt s", rt=nrt, s=16)
                nc.vector.tensor_reduce(fmax[:, col:col + 3], v, axis=AX.XY, op=ALU.max)

            t0 = apool.tile([128, nf], F32, tag="t0")
            t1 = apool.tile([128, nf], F32, tag="t1")
            mu = apool.tile([128, nf], F32, tag="mu")
            e2 = apool.tile([128, nf], F32, tag="e2")

            # ch0: x
            pool_channel(x_core[:], 0)
            # mu = s1/32 - x
            nc.vector.tensor_scalar_mul(t0[:], s1_all[:], 1.0 / KNN)   # s1m
            nc.vector.tensor_tensor(mu[:], t0[:], x_core[:], op=ALU.subtract)
            pool_channel(mu[:], 3)
            # rmax' = rmax - x ; rmin' = rmin - x  (reuse rmax_all/rmin_all in place)
            nc.vector.tensor_tensor(rmax_all[:], rmax_all[:], x_core[:], op=ALU.subtract)
            pool_channel(rmax_all[:], 6)
            nc.vector.tensor_tensor(rmin_all[:], rmin_all[:], x_core[:], op=ALU.subtract)
            pool_channel(rmin_all[:], 9)
            # e2 = mean(rel^2) = s2/32 - x*(mu + s1m)
            nc.vector.tensor_tensor(t1[:], mu[:], t0[:], op=ALU.add)
            nc.vector.tensor_tensor(t1[:], t1[:], x_core[:], op=ALU.mult)
            nc.vector.tensor_scalar_mul(e2[:], s2_all[:], 1.0 / KNN)
            nc.vector.tensor_tensor(e2[:], e2[:], t1[:], op=ALU.subtract)
            pool_channel(e2[:], 27)
            # r_std = sqrt(relu(e2 - mu^2))
            nc.vector.tensor_tensor(t0[:], mu[:], mu[:], op=ALU.mult)  # mu^2
            nc.vector.tensor_tensor(t1[:], e2[:], t0[:], op=ALU.subtract)
            nc.vector.tensor_scalar_max(t1[:], t1[:], 0.0)
            nc.scalar.activation(t1[:], t1[:], ACTF.Sqrt)
            pool_channel(t1[:], 12)
            # x - mu
            nc.vector.tensor_tensor(t1[:], x_core[:], mu[:], op=ALU.subtract)
            pool_channel(t1[:], 15)
            # unit_mu = mu / (|mu| + eps); t0 = mu^2; overwrites mu in place
            nrm = apool.tile([128, nf // 3], F32, tag="nrm")
            nc.vector.tensor_reduce(
                nrm[:], t0[:].rearrange("p (rs c) -> p rs c", c=3),
                axis=AX.X, op=ALU.add,
            )
            nc.scalar.activation(nrm[:], nrm[:], ACTF.Sqrt)
            nc.vector.tensor_scalar_add(nrm[:], nrm[:], EPS)
            nc.vector.reciprocal(nrm[:], nrm[:])
            nc.vector.tensor_tensor(
                mu[:].rearrange("p (rs c) -> p rs c", c=3),
                mu[:].rearrange("p (rs c) -> p rs c", c=3),
                nrm[:].unsqueeze(2).broadcast_to([128, nf // 3, 3]),
                op=ALU.mult,
            )
            umu = mu  # mu now holds unit_mu
            pool_channel(umu[:], 18)
            # cross(x, unit_mu): c_i = x_{i+1} u_{i+2} - x_{i+2} u_{i+1} (mod 3)
            cr = e2  # e2 dead; reuse as cross buffer

            def coord(t, c):
                return t[:].rearrange("p (rs c) -> p rs c", c=3)[:, :, c:c + 1]

            for i in range(3):
                a, bb = (i + 1) % 3, (i + 2) % 3
                nc.vector.tensor_tensor(coord(cr, i), coord(x_core, a), coord(umu, bb), op=ALU.mult)
                nc.vector.tensor_tensor(coord(t0, i), coord(x_core, bb), coord(umu, a), op=ALU.mult)
            nc.vector.tensor_tensor(cr[:], cr[:], t0[:], op=ALU.subtract)
            pool_channel(cr[:], 21)
            # max(rel^2) = max(rmax'^2, rmin'^2)
            nc.vector.tensor_tensor(t0[:], rmax_all[:], rmax_all[:], op=ALU.mult)
            nc.vector.tensor_tensor(t1[:], rmin_all[:], rmin_all[:], op=ALU.mult)
            nc.vector.tensor_tensor(t0[:], t0[:], t1[:], op=ALU.max)
            pool_channel(t0[:], 24)

            # ---- max-pool across partitions, then linear ----
            nc.gpsimd.partition_all_reduce(fmax[:], fmax[:], 128, bass_isa.ReduceOp.max)
            pooledT = apool.tile([30, 1], F32, tag="pooledT")
            nc.sync.dma_start(pool_scratch[:, :], fmax[0:1, 0:30])
            nc.sync.dma_start(pooledT[:], bass.AP(pool_scratch, 0, [[1, 30], [1, 1]]))
            ps = psum.tile([128, 2048], F32, tag="ps")
            nc.tensor.matmul(ps[0:1, 0:32], pooledT[:], wT[:], start=True, stop=True)
            osb = apool.tile([1, 32], F32, tag="osb")
            nc.vector.tensor_tensor(osb[:], ps[0:1, 0:32], brow[:], op=ALU.add)
            nc.sync.dma_start(out_d[bi:bi + 1, :], osb[:])

    return nc


_NC = None


def kernel(x: np.ndarray, W: np.ndarray, b: np.ndarray) -> np.ndarray:
    global _NC
    if _NC is None:
        _NC = build_kernel()
        _NC.finalize()
    nc = _NC
    in_maps = []
    for c in range(NCORES):
        in_maps.append({
            "xs": np.ascontiguousarray(x[c * BPC:(c + 1) * BPC]).astype(np.float32),
            "w": np.ascontiguousarray(W).astype(np.float32),
            "bias": np.ascontiguousarray(b).reshape(1, 32).astype(np.float32),
        })
    res = run_bass_kernel_spmd(nc, in_maps, core_ids=list(range(NCORES)))
    return np.concatenate([r["out"] for r in res.results], axis=0)


if __name__ == "__main__":
    rng = np.random.default_rng(0)
    x = rng.standard_normal((B, N, C), dtype=np.float32)
    W = rng.standard_normal((32, 30), dtype=np.float32) * 0.1
    b = np.zeros(32, dtype=np.float32)
    print(kernel(x, W, b))



# revision 42
# speedup vs baseline: 3.4808x; 3.4808x over previous
"""Trainium2 Bass kernel for nn_Net_87076166960184 (retrieval_knn).

Reference computation per batch b of 16:
  d[n,m]   = |x_n - x_m|^2 over N=4096 points in R^3
  idx      = 32 nearest neighbors per point (incl. self)
  features = 30 channels of neighbor statistics per point
  out[b]   = max-pool_n(features) @ W.T + b          -> [16, 32]

Strategy (pure data parallelism, 2 batches per core):
  - PE computes closeness scores e[n,m] = 2<x_n,x_m> - |x_m|^2 per 128-row
    tile (K=4 matmul; larger = closer; row max = self).
  - Top-32 per row via chunked selection: 16 chunks of 256; per chunk
    Max8 + MaxIndex (exact f32, first-match-dedup) -> 128 candidates.
    Candidates are packed as (value & ~0x1FF) | local_idx so the merge
    phase (4 rounds of Max8/MaxIndex/MatchReplace on the 128-wide packed
    array) orders at 14-mantissa-bit precision and carries the identity
    in the low bits: global = (slot >> 3) * CW + (bits & 0x1FF).
    This does ~21 narrow DVE passes instead of 11 full 4096-wide ones.
    All chunk Max8s are emitted before all MaxIndexes, and stat reduces
    of rt-2 are drained between the merge chain's dependent pairs, so
    adjacent instructions are rarely data-dependent (wait-semaphores
    settle during unrelated work).
  - GPSIMD ap_gather with d=3 fetches xyz of the 32 neighbors per row in
    one instruction from an interleaved replicated source (built on-chip
    by PE broadcast instead of a slow element-strided DMA).
  - Stats (sum/max/min/sum-sq over the 32 neighbors) via single 4D-view
    DVE reduces (deferred one iteration to overlap the gather); 30
    feature channels assembled batch-level; max-pool via free-dim reduce
    + partition_all_reduce; tiny linear on PE.
"""

import os
from contextlib import ExitStack

import numpy as np

import concourse.bacc as bacc
import concourse.bass as bass
import concourse.mybir as mybir
import concourse.tile as tile
from concourse import bass_isa
from concourse.bass_utils import run_bass_kernel_spmd

F32 = mybir.dt.float32
U16 = mybir.dt.uint16
U32 = mybir.dt.uint32
I16 = mybir.dt.int16
AX = mybir.AxisListType
ALU = mybir.AluOpType
ACTF = mybir.ActivationFunctionType

B, N, C = 16, 4096, 3
NCORES = 8
BPC = B // NCORES          # batches per core
KNN = 32
EPS = 1e-8
NEG = -3.0e38
NCHUNK = 16                # selection chunks per row
CW = N // NCHUNK           # chunk width (256); local index fits 9 bits


def build_kernel(n_points=N, bpc=BPC, reps=1, stats_on_pool=False,
                 asm_on_pool=False, no_gather=False, small_gather=False):
    nrt = n_points // 128                  # row tiles per batch
    xel = n_points * C                     # elements of one batch's x

    nc = bacc.Bacc("TRN2", target_bir_lowering=False, debug=False)
    x_in = nc.dram_tensor("xs", [bpc, n_points, C], F32, kind="ExternalInput")
    w_in = nc.dram_tensor("w", [32, 30], F32, kind="ExternalInput")
    b_in = nc.dram_tensor("bias", [1, 32], F32, kind="ExternalInput")
    out_d = nc.dram_tensor("out", [bpc, 32], F32, kind="ExternalOutput")
    pool_scratch = nc.dram_tensor("pool_scratch", [1, 30], F32)

    with tile.TileContext(nc) as tc, ExitStack() as ctx:
        psum = ctx.enter_context(tc.tile_pool(name="psum", bufs=2, space="PSUM"))
        const = ctx.enter_context(tc.tile_pool(name="const", bufs=1))
        epool = ctx.enter_context(tc.tile_pool(name="ebuf", bufs=2))
        spool = ctx.enter_context(tc.tile_pool(name="small", bufs=2))
        gpool = ctx.enter_context(tc.tile_pool(name="gath", bufs=3))
        apool = ctx.enter_context(tc.tile_pool(name="asm", bufs=1))

        wT = const.tile([30, 32], F32, tag="wT")
        nc.sync.dma_start(wT[:], bass.AP(w_in, 0, [[1, 30], [30, 32]]))
        brow = const.tile([1, 32], F32, tag="brow")
        nc.sync.dma_start(brow[:], b_in[:, :])

        ones3 = const.tile([3, 1], F32, tag="ones3")
        nc.vector.memset(ones3[:], 1.0)
        # broadcast lhsT rows at partitions 0/32/64 (matmul base-partition rule)
        ones65 = const.tile([65, 128], F32, tag="ones65")
        for c in range(3):
            nc.vector.memset(ones65[32 * c:32 * c + 1, :], 1.0)
        negrow = const.tile([1, 128], F32, tag="negrow")
        nc.vector.memset(negrow[:], -1.0)

        # lhsTall rows: [2x0, 2x1, 2x2, -1]; row 3 constant across batches
        lhsTall = const.tile([4, n_points], F32, tag="lhsTall")
        for j in range(n_points // 128):
            nc.sync.dma_start(lhsTall[3:4, j * 128:(j + 1) * 128], negrow[:])

        for rep_bi in range(reps * bpc):
            bi = rep_bi % bpc
            xoff = bi * xel
            # rhs4 rows: [x0, x1, x2, sq]
            rhs4 = const.tile([4, n_points], F32, tag="rhs4")
            nc.sync.dma_start(rhs4[0:3, :],
                              bass.AP(x_in, xoff, [[1, 3], [3, n_points]]))
            # bc3: coord rows staged at partitions 0/32/64 (for the PE
            # broadcast below); partition 96 is the sq staging row.
            bc3 = const.tile([97, n_points], F32, tag="bc3")
            for c in range(3):
                nc.sync.dma_start(
                    bc3[32 * c:32 * c + 1, :],
                    bass.AP(x_in, xoff + c, [[0, 1], [3, n_points]]))
            nc.scalar.mul(lhsTall[0:3, :], rhs4[0:3, :], 2.0)
            xsq = epool.tile([128, n_points], F32, tag="e")
            nc.scalar.activation(xsq[0:3, :], rhs4[0:3, :], ACTF.Square)
            for h in range(n_points // 2048):
                ps = psum.tile([128, 2048], F32, tag="ps")
                for j in range(4):
                    ch = slice(h * 2048 + j * 512, h * 2048 + (j + 1) * 512)
                    nc.tensor.matmul(
                        ps[0:1, j * 512:(j + 1) * 512], ones3[:], xsq[0:3, ch],
                        start=True, stop=True,
                    )
                nc.scalar.copy(bc3[96:97, h * 2048:(h + 1) * 2048],
                               ps[0:1, 0:2048])
                nc.sync.dma_start(rhs4[3:4, h * 2048:(h + 1) * 2048],
                                  bc3[96:97, h * 2048:(h + 1) * 2048])

            # xyz_inter[p, 3m+c] = x[m, c] replicated on all partitions,
            # built by PE broadcast (ones^T @ row) instead of strided DMA.
            xyz_inter = const.tile([128, n_points * 3], F32, tag="xyz_inter")
            xyz_v = xyz_inter[:].rearrange("p (m c) -> p c m", c=3)
            for c in range(3):
                for h in range(n_points // 2048):
                    ps = psum.tile([128, 2048], F32, tag="ps")
                    for j in range(4):
                        ch = slice(h * 2048 + j * 512, h * 2048 + (j + 1) * 512)
                        nc.tensor.matmul(
                            ps[:, j * 512:(j + 1) * 512],
                            ones65[32 * c:32 * c + 1, :],
                            bc3[32 * c:32 * c + 1, ch], start=True, stop=True,
                        )
                    nc.scalar.copy(
                        xyz_v[:, c, h * 2048:(h + 1) * 2048], ps[:, 0:2048])

            # x in (core,slot) layout: [128, nrt, 48],
            # x_core[16k+p, rt, s*3+c] = x[rt*128+16k+s, c]
            x_core = const.tile([128, nrt * 48], F32, tag="x_core")
            for k in range(8):
                nc.sync.dma_start(
                    x_core[16 * k:16 * (k + 1), :].rearrange(
                        "p (rt sc) -> p rt sc", sc=48),
                    bass.AP(x_in, xoff + 16 * k * 3, [[0, 16], [384, nrt], [1, 48]]),
                )

            # ---- per-batch stat accumulators, layout (rt, s, c) ----
            s1_all = const.tile([128, nrt * 48], F32, tag="s1_all")
            s2_all = const.tile([128, nrt * 48], F32, tag="s2_all")
            rmax_all = const.tile([128, nrt * 48], F32, tag="rmax_all")
            rmin_all = const.tile([128, nrt * 48], F32, tag="rmin_all")

            # ---- main loop over row tiles.  Stats of rt-2 are queued as
            # thunks and drained one-at-a-time between data-dependent pairs
            # of the merge chain: each drained op is independent work that
            # absorbs the semaphore-settle bubble of the adjacent dep. ----
            stat_q = []

            def queue_stats(rt, g):
                sl = slice(rt * 48, (rt + 1) * 48)
                # g layout: [p, (j s c)] j=32 s=16 c=3
                gv = g[:].rearrange("p (j s c) -> p s c j", s=16, c=3)

                def slot(acc):
                    return acc[:, sl].rearrange("p (s c) -> p s c", c=3)

                # square on ACT (runs ahead) into scratch shared with asm t0
                gsq = apool.tile([128, KNN * 48], F32, tag="t0")
                nc.scalar.activation(gsq[:], g[:], ACTF.Square)
                sv = gsq[:].rearrange("p (j s c) -> p s c j", s=16, c=3)
                stat_q.extend([
                    lambda: nc.vector.tensor_reduce(
                        slot(s1_all), gv, axis=AX.X, op=ALU.add),
                    lambda: nc.vector.tensor_reduce(
                        slot(rmax_all), gv, axis=AX.X, op=ALU.max),
                    lambda: nc.vector.tensor_reduce(
                        slot(rmin_all), gv, axis=AX.X, op=ALU.min),
                    lambda: nc.vector.tensor_reduce(
                        slot(s2_all), sv, axis=AX.X, op=ALU.add),
                ])

            def drain_stat():
                if stat_q:
                    stat_q.pop(0)()

            pending = []
            for rt in range(nrt):
                ebuf = epool.tile([128, n_points], F32, tag="e")
                for half in range(n_points // 2048):
                    ps = psum.tile([128, 2048], F32, tag="ps")
                    for j in range(4):
                        ch = half * 2048 + j * 512
                        nc.tensor.matmul(
                            ps[:, j * 512:(j + 1) * 512],
                            lhsTall[:, rt * 128:(rt + 1) * 128],
                            rhs4[:, ch:ch + 512],
                            start=True, stop=True,
                        )
                    nc.scalar.copy(
                        ebuf[:, half * 2048:(half + 1) * 2048], ps[:, 0:2048])

                # chunk phase: exact f32 top-8 of each CW-wide chunk.  All
                # Max8 first, then all MaxIndex, so no instruction depends
                # on its immediate predecessor (wait-sems settle for free).
                cand_v = spool.tile([128, 8 * NCHUNK], F32, tag="cand_v")
                cand_l = spool.tile([128, 8 * NCHUNK], U32, tag="cand_l")
                for cch in range(NCHUNK):
                    lo, hi = cch * CW, min((cch + 1) * CW, n_points)
                    nc.vector.max(cand_v[:, cch * 8:(cch + 1) * 8], ebuf[:, lo:hi])
                for cch in range(NCHUNK):
                    lo, hi = cch * CW, min((cch + 1) * CW, n_points)
                    nc.vector.max_index(
                        cand_l[:, cch * 8:(cch + 1) * 8],
                        cand_v[:, cch * 8:(cch + 1) * 8], ebuf[:, lo:hi])
                # pack: candw = (value & ~0x1FF) | local  (9-bit local)
                candw = spool.tile([128, 8 * NCHUNK], U32, tag="candw")
                nc.vector.tensor_scalar(
                    candw[:], cand_v[:].bitcast(U32), 0xFFFFFE00, None,
                    ALU.bitwise_and)
                drain_stat()
                nc.vector.tensor_tensor(
                    candw[:], candw[:], cand_l[:], op=ALU.bitwise_or)
                candp = candw[:].bitcast(F32)

                # merge phase: 4 rounds of top-8 on the packed candidates
                vals = spool.tile([128, 32], F32, tag="vals")
                slots = spool.tile([128, 32], U16, tag="slots")
                for it in range(4):
                    v8 = vals[:, it * 8:(it + 1) * 8]
                    nc.vector.max(v8, candp)
                    drain_stat()
                    nc.vector.max_index(slots[:, it * 8:(it + 1) * 8], v8, candp)
                    if it < 3:
                        nc.vector.match_replace(candp, v8, candp, NEG)

                # global idx = (slot >> 3) * CW + (bits(val) & 0x1FF)
                idxs = spool.tile([128, 32], U16, tag="idxs")
                loc16 = spool.tile([128, 32], U16, tag="loc16")
                nc.vector.tensor_scalar(
                    loc16[:], vals[:].bitcast(U16)[:, 0::2], 0x1FF, None,
                    ALU.bitwise_and)
                nc.vector.tensor_scalar(
                    idxs[:], slots[:], 3, None, ALU.logical_shift_right)
                nc.vector.tensor_scalar(
                    idxs[:], idxs[:], CW, None, ALU.mult)
                nc.vector.tensor_tensor(
                    idxs[:], idxs[:], loc16[:], op=ALU.add)

                # gather xyz of the 32 neighbors of the group's 16 rows:
                # g[16k+p, (16j+s)*3+c] = x[nbr_j(row 16k+s), c]
                g = gpool.tile([128, KNN * 48], F32, tag="g")
                if no_gather:  # timing-attribution variant: wrong results
                    nc.gpsimd.memset(g[:], 1.0)
                elif small_gather:  # attribution: 1/4-size source, wrong results
                    nc.vector.tensor_scalar(
                        idxs[:], idxs[:], 0x3FF, None, ALU.bitwise_and)
                    nc.gpsimd.ap_gather(
                        g[:], xyz_inter[:, 0:3 * 1024], idxs[:].bitcast(I16),
                        channels=128, num_elems=1024, d=3, num_idxs=KNN * 16)
                else:
                    nc.gpsimd.ap_gather(
                        g[:], xyz_inter[:], idxs[:].bitcast(I16),
                        channels=128, num_elems=n_points, d=3, num_idxs=KNN * 16)
                pending.append((rt, g))
                if len(pending) > 2:
                    queue_stats(*pending.pop(0))
            while pending:
                queue_stats(*pending.pop(0))
            while stat_q:
                drain_stat()

            # ---- feature assembly, batch level ----
            # layout of all [128, nrt*48] stat tensors: (rt, s, c)
            fmax = apool.tile([128, 30], F32, tag="fmax")
            nf = nrt * 48

            pc_eng = nc.gpsimd if asm_on_pool else nc.vector

            def pool_channel(src_ap, col):
                # src_ap: [128, nf] in (rt, s, c) layout; max over (rt, s) -> [128, 3]
                v = src_ap.rearrange("p (rt s c) -> p c rt s", rt=nrt, s=16)
                pc_eng.tensor_reduce(fmax[:, col:col + 3], v, axis=AX.XY, op=ALU.max)

            t0 = apool.tile([128, nf], F32, tag="t0")
            t1 = apool.tile([128, nf], F32, tag="t1")
            mu = apool.tile([128, nf], F32, tag="mu")
            e2 = apool.tile([128, nf], F32, tag="e2")

            # ch0: x
            pool_channel(x_core[:], 0)
            # mu = s1/32 - x
            nc.vector.tensor_scalar_mul(t0[:], s1_all[:], 1.0 / KNN)   # s1m
            nc.vector.scalar_tensor_tensor(
                mu[:], s1_all[:], 1.0 / KNN, x_core[:],
                op0=ALU.mult, op1=ALU.subtract)
            pool_channel(mu[:], 3)
            # rmax' = rmax - x ; rmin' = rmin - x  (reuse rmax_all/rmin_all in place)
            nc.vector.tensor_tensor(rmax_all[:], rmax_all[:], x_core[:], op=ALU.subtract)
            pool_channel(rmax_all[:], 6)
            nc.vector.tensor_tensor(rmin_all[:], rmin_all[:], x_core[:], op=ALU.subtract)
            pool_channel(rmin_all[:], 9)
            # e2 = mean(rel^2) = s2/32 - x*(mu + s1m)
            nc.vector.tensor_tensor(t1[:], mu[:], t0[:], op=ALU.add)
            nc.vector.tensor_tensor(t1[:], t1[:], x_core[:], op=ALU.mult)
            nc.vector.scalar_tensor_tensor(
                e2[:], s2_all[:], 1.0 / KNN, t1[:],
                op0=ALU.mult, op1=ALU.subtract)
            pool_channel(e2[:], 27)
            # r_std = sqrt(relu(e2 - mu^2))
            nc.vector.tensor_tensor(t0[:], mu[:], mu[:], op=ALU.mult)  # mu^2
            nc.vector.tensor_tensor(t1[:], e2[:], t0[:], op=ALU.subtract)
            nc.vector.tensor_scalar_max(t1[:], t1[:], 0.0)
            nc.scalar.activation(t1[:], t1[:], ACTF.Sqrt)
            pool_channel(t1[:], 12)
            # x - mu
            nc.vector.tensor_tensor(t1[:], x_core[:], mu[:], op=ALU.subtract)
            pool_channel(t1[:], 15)
            # unit_mu = mu / (|mu| + eps); t0 = mu^2; overwrites mu in place
            nrm = apool.tile([128, nf // 3], F32, tag="nrm")
            nc.vector.tensor_reduce(
                nrm[:], t0[:].rearrange("p (rs c) -> p rs c", c=3),
                axis=AX.X, op=ALU.add,
            )
            nc.scalar.activation(nrm[:], nrm[:], ACTF.Sqrt)
            nc.vector.tensor_scalar_add(nrm[:], nrm[:], EPS)
            nc.vector.reciprocal(nrm[:], nrm[:])
            nc.vector.tensor_tensor(
                mu[:].rearrange("p (rs c) -> p rs c", c=3),
                mu[:].rearrange("p (rs c) -> p rs c", c=3),
                nrm[:].unsqueeze(2).broadcast_to([128, nf // 3, 3]),
                op=ALU.mult,
            )
            umu = mu  # mu now holds unit_mu
            pool_channel(umu[:], 18)
            # cross(x, unit_mu): c_i = x_{i+1} u_{i+2} - x_{i+2} u_{i+1} (mod 3)
            cr = e2  # e2 dead; reuse as cross buffer

            def coord(t, c):
                return t[:].rearrange("p (rs c) -> p rs c", c=3)[:, :, c:c + 1]

            for i in range(3):
                a, bb = (i + 1) % 3, (i + 2) % 3
                nc.vector.tensor_tensor(coord(cr, i), coord(x_core, a), coord(umu, bb), op=ALU.mult)
                nc.vector.tensor_tensor(coord(t0, i), coord(x_core, bb), coord(umu, a), op=ALU.mult)
            nc.vector.tensor_tensor(cr[:], cr[:], t0[:], op=ALU.subtract)
            pool_channel(cr[:], 21)
            # max(rel^2) = max(rmax'^2, rmin'^2)
            nc.vector.tensor_tensor(t0[:], rmax_all[:], rmax_all[:], op=ALU.mult)
            nc.vector.tensor_tensor(t1[:], rmin_all[:], rmin_all[:], op=ALU.mult)
            nc.vector.tensor_tensor(t0[:], t0[:], t1[:], op=ALU.max)
            pool_channel(t0[:], 24)

            # ---- max-pool across partitions, then linear ----
            nc.gpsimd.partition_all_reduce(fmax[:], fmax[:], 128, bass_isa.ReduceOp.max)
            pooledT = apool.tile([30, 1], F32, tag="pooledT")
            nc.sync.dma_start(pool_scratch[:, :], fmax[0:1, 0:30])
            nc.sync.dma_start(pooledT[:], bass.AP(pool_scratch, 0, [[1, 30], [1, 1]]))
            ps = psum.tile([128, 2048], F32, tag="ps")
            nc.tensor.matmul(ps[0:1, 0:32], pooledT[:], wT[:], start=True, stop=True)
            osb = apool.tile([1, 32], F32, tag="osb")
            nc.vector.tensor_tensor(osb[:], ps[0:1, 0:32], brow[:], op=ALU.add)
            nc.sync.dma_start(out_d[bi:bi + 1, :], osb[:])

    return nc


_NC = None


def kernel(x: np.ndarray, W: np.ndarray, b: np.ndarray) -> np.ndarray:
    global _NC
    if _NC is None:
        _NC = build_kernel()
        _NC.finalize()
    nc = _NC
    in_maps = []
    for c in range(NCORES):
        in_maps.append({
            "xs": np.ascontiguousarray(x[c * BPC:(c + 1) * BPC]).astype(np.float32),
            "w": np.ascontiguousarray(W).astype(np.float32),
            "bias": np.ascontiguousarray(b).reshape(1, 32).astype(np.float32),
        })
    res = run_bass_kernel_spmd(nc, in_maps, core_ids=list(range(NCORES)))
    return np.concatenate([r["out"] for r in res.results], axis=0)


if __name__ == "__main__":
    rng = np.random.default_rng(0)
    x = rng.standard_normal((B, N, C), dtype=np.float32)
    W = rng.standard_normal((32, 30), dtype=np.float32) * 0.1
    b = np.zeros(32, dtype=np.float32)
    print(kernel(x, W, b))
